# revision 12
# baseline (speedup 1.0000x reference)
"""Trainium2 Bass kernel for CDAttnBlock.

Reference computation (per batch element b, all in fp32):
    q,k,v   = split(x  @ Wqkv)   heads=12, d=64
    q2,k2,v2= split(x2 @ Wqkv)
    o1 = attn(q, k,  v);  o2 = attn(q2, k2, v2);  o3 = attn(q, k2, v2)
    y_i = merge(o_i) @ Wout + bout

Sharding: pure data-parallel over batch (B=8) across 8 NeuronCores;
each core runs the identical program on its own batch element, no
collectives.

Per-core design:
  - All matmul operands are fp16 (full 1 cycle/row PE rate; fp32 is 4x
    slower, fp32r 3x; fp16 keeps end-to-end rel err ~7e-4). PSUM
    accumulation is fp32.
  - x.T built via PE transposes into one [128, 6*1024] fp16 tile
    (hidden on partitions); q.T/k.T per head-pair [128, 1024]; v per
    s-tile [128, 12*65] with a ones column per head so the a@v matmul
    also emits the softmax denominator row.
  - Scores are computed transposed (sT = kT.T @ qT) so softmax needs no
    on-chip transposes; exp runs on ScalarE straight out of PSUM with
    the 1/sqrt(d) scale folded in; no max-subtraction (scores ~N(0,1)).
  - Denominators from 4 heads are collected at partitions {0,32,64,96},
    inverted in ONE DVE reciprocal (its cost scales only with the free
    size), broadcast to 64 partitions on the idle GpSimd engine
    (partition_broadcast), and applied with a fp16 DVE multiply.
  - o accumulates transposed so the output projection (oT as stationary
    operand) yields y in natural [1024, 768] layout.
  - The attention phases are ScalarE(exp)-bound; leaving the PE idle
    there makes the HAM clock-gate drop it to 1.2 GHz. So all other PE
    work (x2 transposes + its qkv projections, then the output
    projections) is chopped into small thunks and interleaved into the
    attention instruction stream to keep the PE continuously busy.
"""

import numpy as np

import concourse.bass as bass
import concourse.tile as tile
from concourse import bacc, mybir
from concourse.bass_utils import run_bass_kernel_spmd
from concourse.masks import make_identity

F32 = mybir.dt.float32
F16 = mybir.dt.float16
AF = mybir.ActivationFunctionType

HIDDEN = 768
HEADS = 12
D = 64
S = 1024
B = 8
SCALE = D ** -0.5
NPAIR = HEADS // 2          # 6 head pairs
KT = HIDDEN // 128          # 6 k-tiles over hidden
ST = S // 128               # 8 s-tiles
VW = D + 1                  # 65: v columns + ones column


class Ctx:
    """Shared handles for the kernel builder."""


def _emit_xt(c, x_ap, xT, xnat, psum_pool, psum_tag, thunks=None):
    """Build xT [128, KT*S] fp16 from x [S, H]: DMA natural tiles, PE
    transpose 6 blocks per s-tile into one psum tile, one DVE evac."""
    nc = c.nc
    out3 = xT.rearrange("p (h s) -> p h s", s=S)
    xns = {}

    def dma(st):
        def f():
            xn = xnat.tile([128, HIDDEN], F32, name="xn", tag="xn")
            xns[st] = xn
            nc.sync.dma_start(xn[:], x_ap[st * 128:(st + 1) * 128, :])
        return f

    def tp(st, half):
        def f():
            tag = psum_tag[half % len(psum_tag)] if isinstance(
                psum_tag, (list, tuple)) else psum_tag
            pt = psum_pool.tile([128, 3 * 128], F32, name="tpp", tag=tag)
            for i in range(3):
                ht = 3 * half + i
                nc.tensor.transpose(
                    pt[:, i * 128:(i + 1) * 128],
                    xns[st][:, ht * 128:(ht + 1) * 128], c.ident[:])
            nc.vector.tensor_copy(
                out3[:, 3 * half:3 * half + 3, st * 128:(st + 1) * 128],
                pt.rearrange("p (h s) -> p h s", s=128))
        return f

    for st in range(ST):
        for f in (dma(st), tp(st, 0), tp(st, 1)):
            if thunks is None:
                f()
            else:
                thunks.append(f)


def _emit_qkv(c, xT, qT, kT, v_st, psum_pool, psum_tag, thunks=None,
              parts=("v", "q", "k")):
    """xT [128, KT*S] fp16 -> qT/kT per pair [128, S] fp16 and v per
    s-tile [128, 12*65] fp16 (with ones column). `parts` selects which
    of v/q/k to emit."""
    nc = c.nc

    def xts(kt, a, b):
        return xT[:, kt * S + a:kt * S + b]

    # ---- v: out [s-tile, 768] accumulated over kt ----
    def v_half(st, half):
        def f():
            tag = psum_tag[half % len(psum_tag)] if isinstance(
                psum_tag, (list, tuple)) else psum_tag
            lo, hi = (0, 512) if half == 0 else (512, 768)
            vp = psum_pool.tile([128, hi - lo], F32, name="vp", tag=tag)
            for kt in range(KT):
                nc.tensor.matmul(
                    vp[:], xts(kt, st * 128, (st + 1) * 128),
                    c.wq16[kt][:, 2 * HIDDEN + lo:2 * HIDDEN + hi],
                    start=(kt == 0), stop=(kt == KT - 1))
            vs3 = v_st[st].rearrange("p (h w) -> p h w", w=VW)
            ha, hb = (0, 8) if half == 0 else (8, 12)
            nc.vector.tensor_copy(
                vs3[:, ha:hb, 0:D],
                vp.rearrange("p (h w) -> p h w", w=D))
            if half == 1:
                nc.vector.tensor_copy(
                    vs3[:, :, D:VW],
                    c.onescol[:, None, :].broadcast_to([128, HEADS, 1]))
        return f

    if "v" in parts:
        for st in range(ST):
            for half in range(2):
                f = v_half(st, half)
                if thunks is None:
                    f()
                else:
                    thunks.append(f)

    # ---- qT / kT per pair: lhsT = Wq/Wk col slice, rhs = xT ----
    def qk_half(p, base, dst, half, hold):
        def f():
            tag = psum_tag[half % len(psum_tag)] if isinstance(
                psum_tag, (list, tuple)) else psum_tag
            pp = psum_pool.tile([128, 512], F32, name="qkp", tag=tag)
            lo = half * 512
            for kt in range(KT):
                nc.tensor.matmul(
                    pp[:],
                    c.wq16[kt][:, base + p * 128:base + (p + 1) * 128],
                    xts(kt, lo, lo + 512),
                    start=(kt == 0), stop=(kt == KT - 1))
            nc.vector.tensor_copy(dst[p][:, lo:lo + 512], pp[:])
        return f

    sel = [(0, qT)] * ("q" in parts) + [(HIDDEN, kT)] * ("k" in parts)
    for p in range(NPAIR):
        for base, dst in sel:
            hold = {}
            for half in range(2):
                f = qk_half(p, base, dst, half, hold)
                if thunks is None:
                    f()
                else:
                    thunks.append(f)


def _emit_proj(c, oT, y_dram, psum_pool, psum_tag, thunks=None):
    """y = oT.T @ Wout + bias -> DRAM, natural [S, H] layout."""
    nc = c.nc

    def half(st, h, hold):
        def f():
            tag = psum_tag[h % len(psum_tag)] if isinstance(
                psum_tag, (list, tuple)) else psum_tag
            lo, hi = (0, 512) if h == 0 else (512, 768)
            yp = psum_pool.tile([128, hi - lo], F32, name="yp", tag=tag)
            for ct in range(KT):
                nc.tensor.matmul(
                    yp[:], oT[ct][:, st * 128:(st + 1) * 128],
                    c.wout16[ct][:, lo:hi],
                    start=(ct == 0), stop=(ct == KT - 1))
            if h == 0:
                hold["yt"] = c.ysb.tile([128, HIDDEN], F32, name="yt",
                                        tag="yt")
            yt = hold["yt"]
            nc.vector.tensor_add(yt[:, lo:hi], yp[:],
                                 c.bias_sb[:, lo:hi])
            if h == 1:
                nc.sync.dma_start(y_dram[st * 128:(st + 1) * 128, :], yt[:])
        return f

    for st in range(ST):
        hold = {}
        for h in range(2):
            f = half(st, h, hold)
            if thunks is None:
                f()
            else:
                thunks.append(f)


def _attn(c, tc, qT, kT, v_st, oT, thunks, npump=None):
    """One attention (12 heads, one at a time). `thunks` (aux PE work +
    deferred normalize chains) are pumped into the exp-wait gaps at an
    even pace so the PE stays busy (and the HAM clock-gate warm) for the
    whole phase."""
    nc = c.nc
    work = list(thunks)          # PE-heavy aux thunks
    dveq = []                    # deferred normalize (DVE/gpsimd only)
    state = {"i": 0, "credit": 0.0, "j": 0}
    # pump sites: one per kt per head (96) plus one per head tail (12)
    sites = HEADS * ST + HEADS
    rate = None

    def pump(k=1.0):
        nonlocal rate
        if rate is None:
            rate = len(work) / sites
        state["credit"] += k * rate
        while state["credit"] >= 1.0 and state["i"] < len(work):
            work[state["i"]]()
            state["i"] += 1
            state["credit"] -= 1.0

    def pump_dve(n=1):
        for _ in range(n):
            if state["j"] < len(dveq):
                dveq[state["j"]]()
                state["j"] += 1

    def drain():
        while state["i"] < len(work):
            work[state["i"]]()
            state["i"] += 1
        while state["j"] < len(dveq):
            dveq[state["j"]]()
            state["j"] += 1

    sps = tc.alloc_tile_pool(name="sps", bufs=2, space="PSUM")
    ovps = tc.alloc_tile_pool(name="ovps", bufs=1, space="PSUM")
    exps = tc.alloc_tile_pool(name="exps", bufs=4, side="right")
    ovstage = tc.alloc_tile_pool(name="ovstage", bufs=5, side="right")
    smalls = tc.alloc_tile_pool(name="smalls", bufs=2, side="right")

    def normalize_thunks(den4, ovs4, g):
        """Deferred DVE/gpsimd normalize chain for one 4-head group."""
        hold = {}

        def t_recip():
            recf = smalls.tile([128, S], F32, name="recf", tag="recf",
                               bufs=1)
            nc.vector.reciprocal_approx_fast(recf[:], den4[:])
            hold["recf"] = recf

        def t_head(gi):
            def f():
                h = 4 * g + gi
                p, hh = h // 2, h % 2
                hp = slice(hh * D, (hh + 1) * D)
                # rrow stages the recip row at partition 0 (the gpsimd
                # broadcast ucode reads its source from partition 0)
                rrow = smalls.tile([1, S], F16, name="rrow", tag="rrow",
                                   bufs=2)
                nc.vector.tensor_copy(
                    rrow[:], hold["recf"][32 * gi:32 * gi + 1, :])
                bcs = smalls.tile([D, S], F16, name="bcs", tag="bcs",
                                  bufs=2)
                nc.gpsimd.partition_broadcast(bcs[:], rrow[:])
                nc.vector.tensor_mul(oT[p][hp, :], ovs4[gi][:], bcs[:])
            return f

        return [t_recip] + [t_head(gi) for gi in range(4)]

    for g in range(HEADS // 4):       # 3 groups of 4 heads
        den4 = smalls.tile([128, S], F32, name="den4", tag="den4", bufs=1)
        nc.vector.memset(den4[:], 1.0)
        ovs4 = []
        for gi in range(4):
            h = 4 * g + gi
            p, hh = h // 2, h % 2
            hp = slice(hh * D, (hh + 1) * D)
            ov = ovps.tile([VW, S], F32, name="ov", tag="ov")
            for kt in range(ST):
                sp = sps.tile([128, S], F32, name="sp", tag="sp")
                kts = kT[p][hp, kt * 128:(kt + 1) * 128]
                for nb in range(2):
                    nc.tensor.matmul(
                        sp[:, nb * 512:(nb + 1) * 512], kts,
                        qT[p][hp, nb * 512:(nb + 1) * 512],
                        start=True, stop=True)
                ex = exps.tile([128, S], F16, name="ex", tag="ex")
                nc.scalar.activation(ex[:], sp[:], AF.Exp,
                                     bias=c.zbias[:], scale=SCALE)
                vs = v_st[kt].rearrange("q (h w) -> q h w", w=VW)[:, h, :]
                for nb in range(2):
                    nc.tensor.matmul(
                        ov[:, nb * 512:(nb + 1) * 512], vs,
                        ex[:, nb * 512:(nb + 1) * 512],
                        start=(kt == 0), stop=(kt == ST - 1))
                # front-load aux work within the head: the last kts stay
                # clear so the staging copies that release the ov PSUM
                # slot aren't queued behind aux DVE evacs
                pump(4.0 / 3.0 if kt < 6 else 0.0)
                if kt < 3:
                    pump_dve(1)
            nc.vector.tensor_copy(den4[32 * gi:32 * gi + 1, :],
                                  ov[D:VW, :])
            ovs = ovstage.tile([D, S], F16, name="ovs", tag="ovs")
            nc.vector.tensor_copy(ovs[:], ov[0:D, :])
            ovs4.append(ovs)
            pump(1.0)
        dveq.extend(normalize_thunks(den4, ovs4, g))
    drain()
    smalls.release()
    ovstage.release()
    exps.release()
    ovps.release()
    sps.release()


def build_kernel(ctx, tc, x, x2, wq, wo, bo, y1, y2, y3):
    nc = tc.nc
    c = Ctx()
    c.nc = nc

    # ---------------- constants (no DMA) ------------------------------
    const = ctx.enter_context(tc.tile_pool(name="const", bufs=1))
    c.ident = const.tile([128, 128], F32, name="ident")
    make_identity(nc, c.ident)
    c.zbias = const.tile([128, 1], F32, name="zbias")
    nc.vector.memset(c.zbias[:], 0.0)
    c.onescol = const.tile([128, 1], F32, name="onescol")
    nc.vector.memset(c.onescol[:], 1.0)
    c.bias_sb = const.tile([128, HIDDEN], F32, name="bias_sb")

    def persist(pool, shape, base, n, dtype=F16):
        return [pool.tile(shape, dtype, name=f"{base}{i}", tag=f"{base}{i}")
                for i in range(n)]

    # ---- persistent pool allocs (LIFO release order) -----------------
    woutp = ctx.enter_context(tc.tile_pool(name="woutp", bufs=1))
    qxp = ctx.enter_context(tc.tile_pool(name="qxp", bufs=1))
    qT_x = persist(qxp, [128, S], "qTx", NPAIR)
    # x2's qkv allocated before kvxp so kvxp can release first (LIFO)
    kvx2p = ctx.enter_context(tc.tile_pool(name="kvx2p", bufs=1))
    qT_x2 = persist(kvx2p, [128, S], "qTx2", NPAIR)
    kT_x2 = persist(kvx2p, [128, S], "kTx2", NPAIR)
    v_x2 = persist(kvx2p, [128, HEADS * VW], "vx2", ST)
    # oT slots: tag "oTa" holds oT1 then oT3; "oTb" holds oT2
    otp = ctx.enter_context(tc.tile_pool(name="otp", bufs=1))
    oT1 = persist(otp, [128, S], "oTa", NPAIR)
    kvxp = tc.alloc_tile_pool(name="kvxp", bufs=1)
    kT_x = persist(kvxp, [128, S], "kTx", NPAIR)
    v_x = persist(kvxp, [128, HEADS * VW], "vx", ST)
    wqp = tc.alloc_tile_pool(name="wqp", bufs=1)
    x2tp = tc.alloc_tile_pool(name="x2tp", bufs=1)
    x2T = x2tp.tile([128, KT * S], F16, name="x2T")
    # ---------------- phase 1 -----------------------------------------
    # DMA order: x first (transposes start ~4us in and keep the PE HAM
    # warm), then x2 (x2T transposes fill the PE while Wqkv streams),
    # then Wqkv, then bias/Wout (needed only by phase 3). Weight
    # fp32->fp16 conversion runs on ScalarE, which is idle until the
    # first attention exp; DVE keeps the PSUM-evac role.
    xnat = tc.alloc_tile_pool(name="xnat", bufs=4, side="right")
    wstage = tc.alloc_tile_pool(name="wstage", bufs=2, side="right")
    p1ps = tc.alloc_tile_pool(name="p1ps", bufs=2, space="PSUM")
    xtp = tc.alloc_tile_pool(name="xtp", bufs=1)
    xT = xtp.tile([128, KT * S], F16, name="xT")
    _emit_xt(c, x, xT, xnat, p1ps, "p1")
    _emit_xt(c, x2, x2T, xnat, p1ps, "p1")
    c.wq16 = []
    for kt in range(KT):
        f = wstage.tile([128, 3 * HIDDEN], F32, name="wqf", tag="wqf")
        nc.sync.dma_start(f[:], wq[kt * 128:(kt + 1) * 128, :])
        t = wqp.tile([128, 3 * HIDDEN], F16, name=f"wq16{kt}",
                     tag=f"wq16{kt}")
        nc.scalar.copy(t[:], f[:])
        c.wq16.append(t)
    bo_bcast = bass.AP(tensor=bo.tensor, offset=bo.offset,
                       ap=[[0, 128]] + list(bo.ap))
    nc.sync.dma_start(c.bias_sb[:], bo_bcast)
    c.wout16 = []
    for ct in range(KT):
        f = wstage.tile([128, HIDDEN], F32, name="wof", tag="wof")
        nc.sync.dma_start(f[:], wo[ct * 128:(ct + 1) * 128, :])
        t = woutp.tile([128, HIDDEN], F16, name=f"wout{ct}", tag=f"wout{ct}")
        nc.scalar.copy(t[:], f[:])
        c.wout16.append(t)
    wstage.release()
    c.ysb = tc.alloc_tile_pool(name="ysb", bufs=2, side="right")
    _emit_qkv(c, xT, qT_x, kT_x, v_x, p1ps, "p1")
    xtp.release()
    p1ps.release()

    # ---- phase 2: attn(o1), aux = k2 + v2 ----------------------------
    auxp = tc.alloc_tile_pool(name="auxp", bufs=1, space="PSUM")
    AUXT = ["auxA", "auxB"]
    thunks2 = []
    _emit_qkv(c, x2T, qT_x2, kT_x2, v_x2, auxp, AUXT, thunks=thunks2,
              parts=("v", "k"))
    _attn(c, tc, qT_x, kT_x, v_x, oT1, thunks2)

    # ---- phase 3: attn(o3), aux = q2 + proj(y1) ----------------------
    oT3 = persist(otp, [128, S], "oTb", NPAIR)
    thunks3 = []
    _emit_qkv(c, x2T, qT_x2, kT_x2, v_x2, auxp, AUXT, thunks=thunks3,
              parts=("q",))
    _emit_proj(c, oT1, y1, auxp, AUXT, thunks=thunks3)
    _attn(c, tc, qT_x, kT_x2, v_x2, oT3, thunks3)
    x2tp.release()
    wqp.release()
    kvxp.release()

    # ---- phase 4: attn(o2), aux = proj(y3) ---------------------------
    oT2 = persist(otp, [128, S], "oTa", NPAIR)
    thunks4 = []
    _emit_proj(c, oT3, y3, auxp, AUXT, thunks=thunks4)
    _attn(c, tc, qT_x2, kT_x2, v_x2, oT2, thunks4)

    # ---- phase 5: proj(y2) -------------------------------------------
    _emit_proj(c, oT2, y2, auxp, AUXT)
    auxp.release()
    c.ysb.release()
    xnat.release()


def build_bass():
    from contextlib import ExitStack
    nc = bacc.Bacc("TRN2", target_bir_lowering=False, debug=False,
                   num_devices=B)
    x = nc.dram_tensor("x", [S, HIDDEN], F32, kind="ExternalInput").ap()
    x2 = nc.dram_tensor("x2", [S, HIDDEN], F32, kind="ExternalInput").ap()
    wq = nc.dram_tensor("Wqkv", [HIDDEN, 3 * HIDDEN], F32,
                        kind="ExternalInput").ap()
    wo = nc.dram_tensor("Wout", [HIDDEN, HIDDEN], F32,
                        kind="ExternalInput").ap()
    bo = nc.dram_tensor("bout", [HIDDEN], F32, kind="ExternalInput").ap()
    y1 = nc.dram_tensor("y1", [S, HIDDEN], F32, kind="ExternalOutput").ap()
    y2 = nc.dram_tensor("y2", [S, HIDDEN], F32, kind="ExternalOutput").ap()
    y3 = nc.dram_tensor("y3", [S, HIDDEN], F32, kind="ExternalOutput").ap()
    with tile.TileContext(nc) as tc:
        with ExitStack() as ctx:
            build_kernel(ctx, tc, x, x2, wq, wo, bo, y1, y2, y3)
    nc.compile()
    return nc


_NC_CACHE = []


def kernel(x, x2, Wqkv, Wout, bout):
    if not _NC_CACHE:
        _NC_CACHE.append(build_bass())
    nc = _NC_CACHE[0]
    in_maps = [
        {"x": np.ascontiguousarray(x[b]), "x2": np.ascontiguousarray(x2[b]),
         "Wqkv": Wqkv, "Wout": Wout, "bout": bout}
        for b in range(B)
    ]
    res = run_bass_kernel_spmd(nc, in_maps, list(range(B)))
    y1 = np.stack([res.results[b]["y1"] for b in range(B)])
    y2 = np.stack([res.results[b]["y2"] for b in range(B)])
    y3 = np.stack([res.results[b]["y3"] for b in range(B)])
    return (y1, y2, y3)



# revision 19
# speedup vs baseline: 1.0605x; 1.0605x over previous
"""Trainium2 Bass kernel for CDAttnBlock.

Reference computation (per batch element b, all in fp32):
    q,k,v   = split(x  @ Wqkv)   heads=12, d=64
    q2,k2,v2= split(x2 @ Wqkv)
    o1 = attn(q, k,  v);  o2 = attn(q2, k2, v2);  o3 = attn(q, k2, v2)
    y_i = merge(o_i) @ Wout + bout

Sharding: pure data-parallel over batch (B=8) across 8 NeuronCores;
each core runs the identical program on its own batch element, no
collectives.

Per-core design:
  - All matmul operands are fp16 (full 1 cycle/row PE rate; fp32 is 4x
    slower, fp32r 3x; fp16 keeps end-to-end rel err ~7e-4). PSUM
    accumulation is fp32.
  - x.T built via PE transposes into one [128, 6*1024] fp16 tile
    (hidden on partitions); q.T/k.T per head-pair [128, 1024]; v per
    s-tile [128, 12*65] with a ones column per head so the a@v matmul
    also emits the softmax denominator row.
  - Scores are computed transposed (sT = kT.T @ qT) so softmax needs no
    on-chip transposes; exp runs on ScalarE straight out of PSUM with
    the 1/sqrt(d) scale folded in; no max-subtraction (scores ~N(0,1)).
  - Denominators from 4 heads are collected at partitions {0,32,64,96},
    inverted in ONE DVE reciprocal (its cost scales only with the free
    size), broadcast to 64 partitions on the idle GpSimd engine
    (partition_broadcast), and applied with a fp16 DVE multiply.
  - o accumulates transposed so the output projection (oT as stationary
    operand) yields y in natural [1024, 768] layout.
  - The attention phases are ScalarE(exp)-bound; leaving the PE idle
    there makes the HAM clock-gate drop it to 1.2 GHz. So all other PE
    work (x2 transposes + its qkv projections, then the output
    projections) is chopped into small thunks and interleaved into the
    attention instruction stream to keep the PE continuously busy.
"""

import numpy as np

import concourse.bass as bass
import concourse.tile as tile
from concourse import bacc, mybir
from concourse.bass_utils import run_bass_kernel_spmd
from concourse.masks import make_identity

F32 = mybir.dt.float32
F16 = mybir.dt.float16
AF = mybir.ActivationFunctionType

HIDDEN = 768
HEADS = 12
D = 64
S = 1024
B = 8
SCALE = D ** -0.5
NPAIR = HEADS // 2          # 6 head pairs
KT = HIDDEN // 128          # 6 k-tiles over hidden
ST = S // 128               # 8 s-tiles


class Ctx:
    """Shared handles for the kernel builder."""


def _emit_xt(c, x_ap, xT, xnat, psum_pool, psum_tag, thunks=None):
    """Build xT [128, KT*S] fp16 from x [S, H]: DMA natural tiles, PE
    transpose 6 blocks per s-tile into one psum tile, one DVE evac."""
    nc = c.nc
    out3 = xT.rearrange("p (h s) -> p h s", s=S)
    xns = {}

    def dma(st):
        def f():
            xn = xnat.tile([128, HIDDEN], F32, name="xn", tag="xn")
            xns[st] = xn
            nc.sync.dma_start(xn[:], x_ap[st * 128:(st + 1) * 128, :])
        return f

    def tp(st, half):
        def f():
            tag = psum_tag[half % len(psum_tag)] if isinstance(
                psum_tag, (list, tuple)) else psum_tag
            pt = psum_pool.tile([128, 3 * 128], F32, name="tpp", tag=tag)
            for i in range(3):
                ht = 3 * half + i
                nc.tensor.transpose(
                    pt[:, i * 128:(i + 1) * 128],
                    xns[st][:, ht * 128:(ht + 1) * 128], c.ident[:])
            nc.vector.tensor_copy(
                out3[:, 3 * half:3 * half + 3, st * 128:(st + 1) * 128],
                pt.rearrange("p (h s) -> p h s", s=128))
        return f

    for st in range(ST):
        for f in (dma(st), tp(st, 0), tp(st, 1)):
            if thunks is None:
                f()
            else:
                thunks.append(f)


def _emit_qkv(c, xT, qT, kT, v_st, psum_pool, psum_tag, thunks=None,
              parts=("v", "q", "k")):
    """xT [128, KT*S] fp16 -> qT/kT per pair [128, S] fp16 and v per
    s-tile [128, 12*64] fp16. `parts` selects which of v/q/k to emit."""
    nc = c.nc

    def xts(kt, a, b):
        return xT[:, kt * S + a:kt * S + b]

    # ---- v: out [s-tile, 768] accumulated over kt ----
    def v_half(st, half):
        def f():
            tag = psum_tag[half % len(psum_tag)] if isinstance(
                psum_tag, (list, tuple)) else psum_tag
            lo, hi = (0, 512) if half == 0 else (512, 768)
            vp = psum_pool.tile([128, hi - lo], F32, name="vp", tag=tag)
            for kt in range(KT):
                nc.tensor.matmul(
                    vp[:], xts(kt, st * 128, (st + 1) * 128),
                    c.wq16[kt][:, 2 * HIDDEN + lo:2 * HIDDEN + hi],
                    start=(kt == 0), stop=(kt == KT - 1))
            nc.vector.tensor_copy(v_st[st][:, lo:hi], vp[:])
        return f

    if "v" in parts:
        for st in range(ST):
            for half in range(2):
                f = v_half(st, half)
                if thunks is None:
                    f()
                else:
                    thunks.append(f)

    # ---- qT / kT per pair: lhsT = Wq/Wk col slice, rhs = xT ----
    def qk_half(p, base, dst, half, hold):
        def f():
            tag = psum_tag[half % len(psum_tag)] if isinstance(
                psum_tag, (list, tuple)) else psum_tag
            pp = psum_pool.tile([128, 512], F32, name="qkp", tag=tag)
            lo = half * 512
            for kt in range(KT):
                nc.tensor.matmul(
                    pp[:],
                    c.wq16[kt][:, base + p * 128:base + (p + 1) * 128],
                    xts(kt, lo, lo + 512),
                    start=(kt == 0), stop=(kt == KT - 1))
            nc.vector.tensor_copy(dst[p][:, lo:lo + 512], pp[:])
        return f

    sel = [(0, qT)] * ("q" in parts) + [(HIDDEN, kT)] * ("k" in parts)
    for p in range(NPAIR):
        for base, dst in sel:
            hold = {}
            for half in range(2):
                f = qk_half(p, base, dst, half, hold)
                if thunks is None:
                    f()
                else:
                    thunks.append(f)


def _emit_proj(c, oT, y_dram, psum_pool, psum_tag, thunks=None):
    """y = oT.T @ Wout + bias -> DRAM, natural [S, H] layout."""
    nc = c.nc

    def half(st, h, hold):
        def f():
            tag = psum_tag[h % len(psum_tag)] if isinstance(
                psum_tag, (list, tuple)) else psum_tag
            lo, hi = (0, 512) if h == 0 else (512, 768)
            yp = psum_pool.tile([128, hi - lo], F32, name="yp", tag=tag)
            for ct in range(KT):
                nc.tensor.matmul(
                    yp[:], oT[ct][:, st * 128:(st + 1) * 128],
                    c.wout16[ct][:, lo:hi],
                    start=(ct == 0), stop=(ct == KT - 1))
            if h == 0:
                hold["yt"] = c.ysb.tile([128, HIDDEN], F32, name="yt",
                                        tag="yt")
            yt = hold["yt"]
            nc.vector.tensor_add(yt[:, lo:hi], yp[:],
                                 c.bias_sb[:, lo:hi])
            if h == 1:
                nc.sync.dma_start(y_dram[st * 128:(st + 1) * 128, :], yt[:])
        return f

    for st in range(ST):
        hold = {}
        for h in range(2):
            f = half(st, h, hold)
            if thunks is None:
                f()
            else:
                thunks.append(f)


def _attn(c, tc, qT, kT, v_st, oT, thunks, npump=None):
    """One attention, processed as 6 head PAIRS (even head at partitions
    0-63, odd at 64-127 of every tile).

    Per (pair, kt) round: scores+exp for head A then head B keep ScalarE
    100% fed (sps double-buffer); the AV for round kt is emitted one
    round later (both ex tiles then exist) as 3 array passes:
      - v pass nb0: A's v at array cols 0-63, B's at 64-127 (col-tiled,
        concurrent, auto tile_position from out base partitions 0/64)
      - v pass nb1: same
      - den pass: 4 ones-column matmuls col-tiled at out partitions
        {0:A-nb0, 32:B-nb0, 64:A-nb1, 96:B-nb1} of a 1-bank den tile,
        all concurrent; PSUM accumulates the softmax denominators.
    This cuts AV PE time 25% vs per-head 65-wide stationaries and frees
    2 PSUM banks.  `thunks` (aux PE work) are pumped into the ScalarE
    shadow at an even pace so the PE stays busy (HAM warm)."""
    nc = c.nc
    work = list(thunks)          # PE-heavy aux thunks
    dveq = []                    # deferred normalize (DVE/gpsimd only)
    state = {"i": 0, "credit": 0.0, "j": 0}
    sites = NPAIR * ST + NPAIR
    rate = None

    def pump(k=1.0):
        nonlocal rate
        if rate is None:
            rate = len(work) / sites
        state["credit"] += k * rate
        while state["credit"] >= 1.0 and state["i"] < len(work):
            work[state["i"]]()
            state["i"] += 1
            state["credit"] -= 1.0

    def pump_dve(n=1):
        for _ in range(n):
            if state["j"] < len(dveq):
                dveq[state["j"]]()
                state["j"] += 1

    def drain():
        while state["i"] < len(work):
            work[state["i"]]()
            state["i"] += 1
        while state["j"] < len(dveq):
            dveq[state["j"]]()
            state["j"] += 1

    sps = tc.alloc_tile_pool(name="sps", bufs=2, space="PSUM")
    ovps = tc.alloc_tile_pool(name="ovps", bufs=1, space="PSUM")
    denps = tc.alloc_tile_pool(name="denps", bufs=1, space="PSUM")
    exps = tc.alloc_tile_pool(name="exps", bufs=4, side="right")
    ovstage = tc.alloc_tile_pool(name="ovstage", bufs=2, side="right")
    smalls = tc.alloc_tile_pool(name="smalls", bufs=2, side="right")

    def make_avden(pr, kt, ovp, den, exA, exB):
        def f():
            vA = v_st[kt][:, (2 * pr) * D:(2 * pr + 1) * D]
            vB = v_st[kt][:, (2 * pr + 1) * D:(2 * pr + 2) * D]
            st0, st1 = (kt == 0), (kt == ST - 1)
            for nb in range(2):
                sl = slice(nb * 512, (nb + 1) * 512)
                nc.tensor.matmul(ovp[0:D, sl], vA, exA[:, sl],
                                 start=st0, stop=st1)
                nc.tensor.matmul(ovp[D:2 * D, sl], vB, exB[:, sl],
                                 start=st0, stop=st1)
            nc.tensor.matmul(den[0:1, :], c.ones16[:], exA[:, 0:512],
                             start=st0, stop=st1, tile_position=(0, 0))
            nc.tensor.matmul(den[32:33, :], c.ones16[:], exB[:, 0:512],
                             start=st0, stop=st1, tile_position=(0, 32))
            nc.tensor.matmul(den[64:65, :], c.ones16[:], exA[:, 512:1024],
                             start=st0, stop=st1, tile_position=(0, 64))
            nc.tensor.matmul(den[96:97, :], c.ones16[:], exB[:, 512:1024],
                             start=st0, stop=st1, tile_position=(0, 96))
        return f

    def normalize_thunks(pr, ovs2, recf_hold):
        """Deferred DVE/gpsimd normalize for one pair: per head build a
        [64, S] f16 recip-broadcast tile, then one mul into oT.  All
        TensorTensor inputs sit at base partition 0 (verifier rule)."""
        def t_head(hh):
            def f():
                recf = recf_hold["recf"]
                hp = slice(hh * D, (hh + 1) * D)
                bcs = smalls.tile([D, S], F16, name="bcs", tag="bcs",
                                  bufs=2)
                for nb in range(2):
                    # den rows: 0=A-nb0, 32=B-nb0, 64=A-nb1, 96=B-nb1
                    r = 32 * hh + 64 * nb
                    rrow = smalls.tile([1, 512], F16, name="rrow",
                                       tag="rrow", bufs=4)
                    nc.vector.tensor_copy(rrow[:], recf[r:r + 1, :])
                    nc.gpsimd.partition_broadcast(
                        bcs[:, nb * 512:(nb + 1) * 512], rrow[:])
                nc.vector.tensor_mul(oT[pr][hp, :], ovs2[hh][:], bcs[:])
            return f

        return [t_head(0), t_head(1)]

    def stage_pair(pr, ovp, den):
        """Emit at pair end: free ovp + den PSUM (DVE), defer the rest."""
        ovs2 = []
        for hh in range(2):
            ovs = ovstage.tile([D, S], F16, name="ovs", tag="ovs", bufs=4)
            nc.vector.tensor_copy(ovs[:], ovp[hh * D:(hh + 1) * D, :])
            ovs2.append(ovs)
        recf = smalls.tile([128, 512], F32, name="recf", tag="recf",
                           bufs=2)
        nc.vector.reciprocal_approx_fast(recf[:], den[:])
        dveq.extend(normalize_thunks(pr, ovs2, {"recf": recf}))

    pend = None                  # previous round's deferred AV+den
    pend_tail = None             # pair-end staging to emit after it
    for pr in range(NPAIR):
        ovp = ovps.tile([128, S], F32, name="ovp", tag="ov")
        den = denps.tile([128, 512], F32, name="den", tag="den")
        for kt in range(ST):
            exAB = []
            for hh in range(2):
                hp = slice(hh * D, (hh + 1) * D)
                sp = sps.tile([128, S], F32, name="sp", tag="sp")
                kts = kT[pr][hp, kt * 128:(kt + 1) * 128]
                for nb in range(2):
                    nc.tensor.matmul(
                        sp[:, nb * 512:(nb + 1) * 512], kts,
                        qT[pr][hp, nb * 512:(nb + 1) * 512],
                        start=True, stop=True)
                ex = exps.tile([128, S], F16, name="ex", tag="ex")
                nc.scalar.activation(ex[:], sp[:], AF.Exp,
                                     bias=c.zbias[:], scale=SCALE)
                exAB.append(ex)
            if pend is not None:
                pend()
            if pend_tail is not None:
                pend_tail()
                pend_tail = None
            pend = make_avden(pr, kt, ovp, den, exAB[0], exAB[1])
            pump(4.0 / 3.0 if kt < 6 else 0.0)
            if kt < 4:
                pump_dve(1)
        # the kt=7 AV+den runs early in the next pair; its staging
        # (which frees ovp/den for that pair's kt0) follows right after
        pend_tail = (lambda p=pr, o=ovp, d=den: stage_pair(p, o, d))
        pump(1.0)
    pend()
    pend_tail()
    drain()
    smalls.release()
    ovstage.release()
    exps.release()
    denps.release()
    ovps.release()
    sps.release()


def build_kernel(ctx, tc, x, x2, wq, wo, bo, y1, y2, y3):
    nc = tc.nc
    c = Ctx()
    c.nc = nc

    # ---------------- constants (no DMA) ------------------------------
    const = ctx.enter_context(tc.tile_pool(name="const", bufs=1))
    c.ident = const.tile([128, 128], F32, name="ident")
    make_identity(nc, c.ident)
    c.zbias = const.tile([128, 1], F32, name="zbias")
    nc.vector.memset(c.zbias[:], 0.0)
    c.ones16 = const.tile([128, 1], F16, name="ones16")
    nc.vector.memset(c.ones16[:], 1.0)
    c.bias_sb = const.tile([128, HIDDEN], F32, name="bias_sb")

    def persist(pool, shape, base, n, dtype=F16):
        return [pool.tile(shape, dtype, name=f"{base}{i}", tag=f"{base}{i}")
                for i in range(n)]

    # ---- persistent pool allocs (LIFO release order) -----------------
    woutp = ctx.enter_context(tc.tile_pool(name="woutp", bufs=1))
    qxp = ctx.enter_context(tc.tile_pool(name="qxp", bufs=1))
    qT_x = persist(qxp, [128, S], "qTx", NPAIR)
    # x2's qkv allocated before kvxp so kvxp can release first (LIFO)
    kvx2p = ctx.enter_context(tc.tile_pool(name="kvx2p", bufs=1))
    qT_x2 = persist(kvx2p, [128, S], "qTx2", NPAIR)
    kT_x2 = persist(kvx2p, [128, S], "kTx2", NPAIR)
    v_x2 = persist(kvx2p, [128, HEADS * D], "vx2", ST)
    # oT slots: tag "oTa" holds oT1 then oT3; "oTb" holds oT2
    otp = ctx.enter_context(tc.tile_pool(name="otp", bufs=1))
    oT1 = persist(otp, [128, S], "oTa", NPAIR)
    kvxp = tc.alloc_tile_pool(name="kvxp", bufs=1)
    kT_x = persist(kvxp, [128, S], "kTx", NPAIR)
    v_x = persist(kvxp, [128, HEADS * D], "vx", ST)
    wqp = tc.alloc_tile_pool(name="wqp", bufs=1)
    x2tp = tc.alloc_tile_pool(name="x2tp", bufs=1)
    x2T = x2tp.tile([128, KT * S], F16, name="x2T")
    # ---------------- phase 1 -----------------------------------------
    # DMA order: x first (transposes start ~4us in and keep the PE HAM
    # warm), then x2 (x2T transposes fill the PE while Wqkv streams),
    # then Wqkv, then bias/Wout (needed only by phase 3). Weight
    # fp32->fp16 conversion runs on ScalarE, which is idle until the
    # first attention exp; DVE keeps the PSUM-evac role.
    xnat = tc.alloc_tile_pool(name="xnat", bufs=4, side="right")
    wstage = tc.alloc_tile_pool(name="wstage", bufs=2, side="right")
    p1ps = tc.alloc_tile_pool(name="p1ps", bufs=2, space="PSUM")
    xtp = tc.alloc_tile_pool(name="xtp", bufs=1)
    xT = xtp.tile([128, KT * S], F16, name="xT")
    _emit_xt(c, x, xT, xnat, p1ps, "p1")
    _emit_xt(c, x2, x2T, xnat, p1ps, "p1")
    c.wq16 = []
    for kt in range(KT):
        f = wstage.tile([128, 3 * HIDDEN], F32, name="wqf", tag="wqf")
        nc.sync.dma_start(f[:], wq[kt * 128:(kt + 1) * 128, :])
        t = wqp.tile([128, 3 * HIDDEN], F16, name=f"wq16{kt}",
                     tag=f"wq16{kt}")
        nc.scalar.copy(t[:], f[:])
        c.wq16.append(t)
    bo_bcast = bass.AP(tensor=bo.tensor, offset=bo.offset,
                       ap=[[0, 128]] + list(bo.ap))
    nc.sync.dma_start(c.bias_sb[:], bo_bcast)
    c.wout16 = []
    for ct in range(KT):
        f = wstage.tile([128, HIDDEN], F32, name="wof", tag="wof")
        nc.sync.dma_start(f[:], wo[ct * 128:(ct + 1) * 128, :])
        t = woutp.tile([128, HIDDEN], F16, name=f"wout{ct}", tag=f"wout{ct}")
        nc.scalar.copy(t[:], f[:])
        c.wout16.append(t)
    wstage.release()
    c.ysb = tc.alloc_tile_pool(name="ysb", bufs=2, side="right")
    _emit_qkv(c, xT, qT_x, kT_x, v_x, p1ps, "p1")
    xtp.release()
    p1ps.release()

    # ---- phase 2: attn(o1), aux = k2 + v2 ----------------------------
    auxp = tc.alloc_tile_pool(name="auxp", bufs=1, space="PSUM")
    AUXT = ["aux"]
    thunks2 = []
    _emit_qkv(c, x2T, qT_x2, kT_x2, v_x2, auxp, AUXT, thunks=thunks2,
              parts=("v", "k"))
    _attn(c, tc, qT_x, kT_x, v_x, oT1, thunks2)

    # ---- phase 3: attn(o3), aux = q2 + proj(y1) ----------------------
    oT3 = persist(otp, [128, S], "oTb", NPAIR)
    thunks3 = []
    _emit_qkv(c, x2T, qT_x2, kT_x2, v_x2, auxp, AUXT, thunks=thunks3,
              parts=("q",))
    _emit_proj(c, oT1, y1, auxp, AUXT, thunks=thunks3)
    _attn(c, tc, qT_x, kT_x2, v_x2, oT3, thunks3)
    x2tp.release()
    wqp.release()
    kvxp.release()

    # ---- phase 4: attn(o2), aux = proj(y3) ---------------------------
    oT2 = persist(otp, [128, S], "oTa", NPAIR)
    thunks4 = []
    _emit_proj(c, oT3, y3, auxp, AUXT, thunks=thunks4)
    _attn(c, tc, qT_x2, kT_x2, v_x2, oT2, thunks4)

    # ---- phase 5: proj(y2) -------------------------------------------
    _emit_proj(c, oT2, y2, auxp, AUXT)
    auxp.release()
    c.ysb.release()
    xnat.release()


def build_bass():
    from contextlib import ExitStack
    nc = bacc.Bacc("TRN2", target_bir_lowering=False, debug=False,
                   num_devices=B)
    x = nc.dram_tensor("x", [S, HIDDEN], F32, kind="ExternalInput").ap()
    x2 = nc.dram_tensor("x2", [S, HIDDEN], F32, kind="ExternalInput").ap()
    wq = nc.dram_tensor("Wqkv", [HIDDEN, 3 * HIDDEN], F32,
                        kind="ExternalInput").ap()
    wo = nc.dram_tensor("Wout", [HIDDEN, HIDDEN], F32,
                        kind="ExternalInput").ap()
    bo = nc.dram_tensor("bout", [HIDDEN], F32, kind="ExternalInput").ap()
    y1 = nc.dram_tensor("y1", [S, HIDDEN], F32, kind="ExternalOutput").ap()
    y2 = nc.dram_tensor("y2", [S, HIDDEN], F32, kind="ExternalOutput").ap()
    y3 = nc.dram_tensor("y3", [S, HIDDEN], F32, kind="ExternalOutput").ap()
    with tile.TileContext(nc) as tc:
        with ExitStack() as ctx:
            build_kernel(ctx, tc, x, x2, wq, wo, bo, y1, y2, y3)
    nc.compile()
    return nc


_NC_CACHE = []


def kernel(x, x2, Wqkv, Wout, bout):
    if not _NC_CACHE:
        _NC_CACHE.append(build_bass())
    nc = _NC_CACHE[0]
    in_maps = [
        {"x": np.ascontiguousarray(x[b]), "x2": np.ascontiguousarray(x2[b]),
         "Wqkv": Wqkv, "Wout": Wout, "bout": bout}
        for b in range(B)
    ]
    res = run_bass_kernel_spmd(nc, in_maps, list(range(B)))
    y1 = np.stack([res.results[b]["y1"] for b in range(B)])
    y2 = np.stack([res.results[b]["y2"] for b in range(B)])
    y3 = np.stack([res.results[b]["y3"] for b in range(B)])
    return (y1, y2, y3)



# revision 22
# speedup vs baseline: 1.1434x; 1.0782x over previous
"""Trainium2 Bass kernel for CDAttnBlock.

Reference computation (per batch element b, all in fp32):
    q,k,v   = split(x  @ Wqkv)   heads=12, d=64
    q2,k2,v2= split(x2 @ Wqkv)
    o1 = attn(q, k,  v);  o2 = attn(q2, k2, v2);  o3 = attn(q, k2, v2)
    y_i = merge(o_i) @ Wout + bout

Sharding: pure data-parallel over batch (B=8) across 8 NeuronCores;
each core runs the identical program on its own batch element, no
collectives.

Per-core design:
  - All matmul operands are fp16 (full 1 cycle/row PE rate; fp32 is 4x
    slower, fp32r 3x; fp16 keeps end-to-end rel err ~7e-4). PSUM
    accumulation is fp32.
  - x.T built via PE transposes into one [128, 6*1024] fp16 tile
    (hidden on partitions); q.T/k.T per head-pair [128, 1024]; v per
    s-tile [128, 12*65] with a ones column per head so the a@v matmul
    also emits the softmax denominator row.
  - Scores are computed transposed (sT = kT.T @ qT) so softmax needs no
    on-chip transposes; exp runs on ScalarE straight out of PSUM with
    the 1/sqrt(d) scale folded in; no max-subtraction (scores ~N(0,1)).
  - Denominators from 4 heads are collected at partitions {0,32,64,96},
    inverted in ONE DVE reciprocal (its cost scales only with the free
    size), broadcast to 64 partitions on the idle GpSimd engine
    (partition_broadcast), and applied with a fp16 DVE multiply.
  - o accumulates transposed so the output projection (oT as stationary
    operand) yields y in natural [1024, 768] layout.
  - The attention phases are ScalarE(exp)-bound; leaving the PE idle
    there makes the HAM clock-gate drop it to 1.2 GHz. So all other PE
    work (x2 transposes + its qkv projections, then the output
    projections) is chopped into small thunks and interleaved into the
    attention instruction stream to keep the PE continuously busy.
"""

import numpy as np

import concourse.bass as bass
import concourse.tile as tile
from concourse import bacc, mybir
from concourse.bass_utils import run_bass_kernel_spmd
from concourse.masks import make_identity

F32 = mybir.dt.float32
F16 = mybir.dt.float16
AF = mybir.ActivationFunctionType

HIDDEN = 768
HEADS = 12
D = 64
S = 1024
B = 8
SCALE = D ** -0.5
NPAIR = HEADS // 2          # 6 head pairs
KT = HIDDEN // 128          # 6 k-tiles over hidden
ST = S // 128               # 8 s-tiles


class Ctx:
    """Shared handles for the kernel builder."""


def _emit_xt(c, x_ap, xT, xnat, psum_pool, psum_tag, thunks=None):
    """Build xT [128, KT*S] fp16 from x [S, H]: DMA natural tiles, PE
    transpose 6 blocks per s-tile into one psum tile, one DVE evac."""
    nc = c.nc
    out3 = xT.rearrange("p (h s) -> p h s", s=S)
    xns = {}

    def dma(st):
        def f():
            xn = xnat.tile([128, HIDDEN], F32, name="xn", tag="xn")
            xns[st] = xn
            nc.sync.dma_start(xn[:], x_ap[st * 128:(st + 1) * 128, :])
        return f

    def tp(st, half):
        def f():
            tag = psum_tag[half % len(psum_tag)] if isinstance(
                psum_tag, (list, tuple)) else psum_tag
            pt = psum_pool.tile([128, 3 * 128], F32, name="tpp", tag=tag)
            for i in range(3):
                ht = 3 * half + i
                nc.tensor.transpose(
                    pt[:, i * 128:(i + 1) * 128],
                    xns[st][:, ht * 128:(ht + 1) * 128], c.ident[:])
            nc.vector.tensor_copy(
                out3[:, 3 * half:3 * half + 3, st * 128:(st + 1) * 128],
                pt.rearrange("p (h s) -> p h s", s=128))
        return f

    for st in range(ST):
        for f in (dma(st), tp(st, 0), tp(st, 1)):
            if thunks is None:
                f()
            else:
                thunks.append(f)


def _emit_qkv(c, xT, qT, kT, v_st, psum_pool, psum_tag, thunks=None,
              parts=("v", "q", "k")):
    """xT [128, KT*S] fp16 -> qT/kT per pair [128, S] fp16 and v per
    s-tile [128, 12*64] fp16. `parts` selects which of v/q/k to emit."""
    nc = c.nc

    def xts(kt, a, b):
        return xT[:, kt * S + a:kt * S + b]

    # ---- v: out [s-tile, 768] accumulated over kt ----
    def v_half(st, half):
        def f():
            tag = psum_tag[half % len(psum_tag)] if isinstance(
                psum_tag, (list, tuple)) else psum_tag
            lo, hi = (0, 512) if half == 0 else (512, 768)
            vp = psum_pool.tile([128, hi - lo], F32, name="vp", tag=tag)
            for kt in range(KT):
                nc.tensor.matmul(
                    vp[:], xts(kt, st * 128, (st + 1) * 128),
                    c.wq16[kt][:, 2 * HIDDEN + lo:2 * HIDDEN + hi],
                    start=(kt == 0), stop=(kt == KT - 1))
            nc.vector.tensor_copy(v_st[st][:, lo:hi], vp[:])
        return f

    if "v" in parts:
        for st in range(ST):
            for half in range(2):
                f = v_half(st, half)
                if thunks is None:
                    f()
                else:
                    thunks.append(f)

    # ---- qT / kT per pair: lhsT = Wq/Wk col slice, rhs = xT ----
    def qk_half(p, base, dst, half, hold):
        def f():
            tag = psum_tag[half % len(psum_tag)] if isinstance(
                psum_tag, (list, tuple)) else psum_tag
            pp = psum_pool.tile([128, 512], F32, name="qkp", tag=tag)
            lo = half * 512
            for kt in range(KT):
                nc.tensor.matmul(
                    pp[:],
                    c.wq16[kt][:, base + p * 128:base + (p + 1) * 128],
                    xts(kt, lo, lo + 512),
                    start=(kt == 0), stop=(kt == KT - 1))
            nc.vector.tensor_copy(dst[p][:, lo:lo + 512], pp[:])
        return f

    sel = [(0, qT)] * ("q" in parts) + [(HIDDEN, kT)] * ("k" in parts)
    for p in range(NPAIR):
        for base, dst in sel:
            hold = {}
            for half in range(2):
                f = qk_half(p, base, dst, half, hold)
                if thunks is None:
                    f()
                else:
                    thunks.append(f)


def _emit_proj(c, oT, y_dram, psum_pool, psum_tag, thunks=None):
    """y = oT.T @ Wout + bias -> DRAM, natural [S, H] layout."""
    nc = c.nc

    def half(st, h, hold):
        def f():
            tag = psum_tag[h % len(psum_tag)] if isinstance(
                psum_tag, (list, tuple)) else psum_tag
            lo, hi = (0, 512) if h == 0 else (512, 768)
            yp = psum_pool.tile([128, hi - lo], F32, name="yp", tag=tag)
            for ct in range(KT):
                nc.tensor.matmul(
                    yp[:], oT[ct][:, st * 128:(st + 1) * 128],
                    c.wout16[ct][:, lo:hi],
                    start=(ct == 0), stop=(ct == KT - 1))
            if h == 0:
                hold["yt"] = c.ysb.tile([128, HIDDEN], F32, name="yt",
                                        tag="yt")
            yt = hold["yt"]
            nc.vector.tensor_add(yt[:, lo:hi], yp[:],
                                 c.bias_sb[:, lo:hi])
            if h == 1:
                nc.sync.dma_start(y_dram[st * 128:(st + 1) * 128, :], yt[:])
        return f

    for st in range(ST):
        hold = {}
        for h in range(2):
            f = half(st, h, hold)
            if thunks is None:
                f()
            else:
                thunks.append(f)


def _attn(c, tc, qT, kT, v_st, oT, thunks, npump=None):
    """One attention, processed as 6 head PAIRS (even head at partitions
    0-63, odd at 64-127 of every tile).

    Per (pair, kt) round: scores+exp for head A then head B keep ScalarE
    100% fed (sps double-buffer); the AV for round kt is emitted one
    round later (both ex tiles then exist) as 3 array passes:
      - v pass nb0: A's v at array cols 0-63, B's at 64-127 (col-tiled,
        concurrent, auto tile_position from out base partitions 0/64)
      - v pass nb1: same
      - den pass: 4 ones-column matmuls col-tiled at out partitions
        {0:A-nb0, 32:B-nb0, 64:A-nb1, 96:B-nb1} of a 1-bank den tile,
        all concurrent; PSUM accumulates the softmax denominators.
    This cuts AV PE time 25% vs per-head 65-wide stationaries and frees
    2 PSUM banks.  `thunks` (aux PE work) are pumped into the ScalarE
    shadow at an even pace so the PE stays busy (HAM warm)."""
    nc = c.nc
    work = list(thunks)          # PE-heavy aux thunks
    dveq = []                    # deferred normalize (DVE/gpsimd only)
    state = {"i": 0, "credit": 0.0, "j": 0}
    sites = NPAIR * ST + NPAIR
    rate = None

    def pump(k=1.0):
        nonlocal rate
        if rate is None:
            rate = len(work) / sites
        state["credit"] += k * rate
        while state["credit"] >= 1.0 and state["i"] < len(work):
            work[state["i"]]()
            state["i"] += 1
            state["credit"] -= 1.0

    def pump_dve(n=1):
        for _ in range(n):
            if state["j"] < len(dveq):
                dveq[state["j"]]()
                state["j"] += 1

    def drain():
        while state["i"] < len(work):
            work[state["i"]]()
            state["i"] += 1
        while state["j"] < len(dveq):
            dveq[state["j"]]()
            state["j"] += 1

    sps = tc.alloc_tile_pool(name="sps", bufs=2, space="PSUM")
    ovps = tc.alloc_tile_pool(name="ovps", bufs=1, space="PSUM")
    denps = tc.alloc_tile_pool(name="denps", bufs=1, space="PSUM")
    exps = tc.alloc_tile_pool(name="exps", bufs=4, side="right")
    ovstage = tc.alloc_tile_pool(name="ovstage", bufs=2, side="right")
    smalls = tc.alloc_tile_pool(name="smalls", bufs=2, side="right")

    def make_avden(pr, kt, ovp, den, exA, exB):
        def f():
            vA = v_st[kt][:, (2 * pr) * D:(2 * pr + 1) * D]
            vB = v_st[kt][:, (2 * pr + 1) * D:(2 * pr + 2) * D]
            st0, st1 = (kt == 0), (kt == ST - 1)
            for nb in range(2):
                sl = slice(nb * 512, (nb + 1) * 512)
                nc.tensor.matmul(ovp[0:D, sl], vA, exA[:, sl],
                                 start=st0, stop=st1)
                nc.tensor.matmul(ovp[D:2 * D, sl], vB, exB[:, sl],
                                 start=st0, stop=st1)
            nc.tensor.matmul(den[0:1, :], c.ones16[:], exA[:, 0:512],
                             start=st0, stop=st1, tile_position=(0, 0))
            nc.tensor.matmul(den[32:33, :], c.ones16[:], exB[:, 0:512],
                             start=st0, stop=st1, tile_position=(0, 32))
            nc.tensor.matmul(den[64:65, :], c.ones16[:], exA[:, 512:1024],
                             start=st0, stop=st1, tile_position=(0, 64))
            nc.tensor.matmul(den[96:97, :], c.ones16[:], exB[:, 512:1024],
                             start=st0, stop=st1, tile_position=(0, 96))
        return f

    def normalize_thunks(pr, ovs2, recf_hold):
        """Deferred DVE/gpsimd normalize for one pair: per head build a
        [64, S] f16 recip-broadcast tile, then one mul into oT.  All
        TensorTensor inputs sit at base partition 0 (verifier rule)."""
        def t_head(hh):
            def f():
                recf = recf_hold["recf"]
                hp = slice(hh * D, (hh + 1) * D)
                bcs = smalls.tile([D, S], F16, name="bcs", tag="bcs",
                                  bufs=2)
                for nb in range(2):
                    # den rows: 0=A-nb0, 32=B-nb0, 64=A-nb1, 96=B-nb1
                    r = 32 * hh + 64 * nb
                    rrow = smalls.tile([1, 512], F16, name="rrow",
                                       tag="rrow", bufs=4)
                    nc.vector.tensor_copy(rrow[:], recf[r:r + 1, :])
                    nc.gpsimd.partition_broadcast(
                        bcs[:, nb * 512:(nb + 1) * 512], rrow[:])
                nc.vector.tensor_mul(oT[pr][hp, :], ovs2[hh][:], bcs[:])
            return f

        return [t_head(0), t_head(1)]

    def stage_pair(pr, ovp, den):
        """Emit at pair end: free ovp + den PSUM (DVE), defer the rest."""
        ovs2 = []
        for hh in range(2):
            ovs = ovstage.tile([D, S], F16, name="ovs", tag="ovs", bufs=4)
            nc.vector.tensor_copy(ovs[:], ovp[hh * D:(hh + 1) * D, :])
            ovs2.append(ovs)
        recf = smalls.tile([128, 512], F32, name="recf", tag="recf",
                           bufs=2)
        nc.vector.reciprocal_approx_fast(recf[:], den[:])
        dveq.extend(normalize_thunks(pr, ovs2, {"recf": recf}))

    pend = None                  # previous round's deferred AV+den
    pend_tail = None             # pair-end staging to emit after it
    for pr in range(NPAIR):
        ovp = ovps.tile([128, S], F32, name="ovp", tag="ov")
        den = denps.tile([128, 512], F32, name="den", tag="den")
        for kt in range(ST):
            exAB = []
            for hh in range(2):
                hp = slice(hh * D, (hh + 1) * D)
                sp = sps.tile([128, S], F32, name="sp", tag="sp")
                kts = kT[pr][hp, kt * 128:(kt + 1) * 128]
                for nb in range(2):
                    nc.tensor.matmul(
                        sp[:, nb * 512:(nb + 1) * 512], kts,
                        qT[pr][hp, nb * 512:(nb + 1) * 512],
                        start=True, stop=True)
                ex = exps.tile([128, S], F16, name="ex", tag="ex")
                nc.scalar.activation(ex[:], sp[:], AF.Exp,
                                     bias=c.zbias[:], scale=SCALE)
                exAB.append(ex)
            if pend is not None:
                pend()
            if pend_tail is not None:
                pend_tail()
                pend_tail = None
            pend = make_avden(pr, kt, ovp, den, exAB[0], exAB[1])
            # no aux near pair boundaries: the boundary's stage+recip
            # burst on DVE would delay the aux psum evac and head-block
            # the PE FIFO right when ScalarE needs fresh scores
            pump(8.0 / 5.0 if 1 <= kt < 6 else 0.0)
            if 2 <= kt < 6:
                pump_dve(1)
        # the kt=7 AV+den runs early in the next pair; its staging
        # (which frees ovp/den for that pair's kt0) follows right after
        pend_tail = (lambda p=pr, o=ovp, d=den: stage_pair(p, o, d))
        pump(1.0)
    pend()
    pend_tail()
    drain()
    smalls.release()
    ovstage.release()
    exps.release()
    denps.release()
    ovps.release()
    sps.release()


def build_kernel(ctx, tc, x, x2, wq, wo, bo, y1, y2, y3):
    nc = tc.nc
    c = Ctx()
    c.nc = nc

    # ---------------- constants (no DMA) ------------------------------
    const = ctx.enter_context(tc.tile_pool(name="const", bufs=1))
    c.ident = const.tile([128, 128], F32, name="ident")
    make_identity(nc, c.ident)
    c.zbias = const.tile([128, 1], F32, name="zbias")
    nc.vector.memset(c.zbias[:], 0.0)
    c.ones16 = const.tile([128, 1], F16, name="ones16")
    nc.vector.memset(c.ones16[:], 1.0)
    c.bias_sb = const.tile([128, HIDDEN], F32, name="bias_sb")

    def persist(pool, shape, base, n, dtype=F16):
        return [pool.tile(shape, dtype, name=f"{base}{i}", tag=f"{base}{i}")
                for i in range(n)]

    # ---- persistent pool allocs (LIFO release order) -----------------
    woutp = ctx.enter_context(tc.tile_pool(name="woutp", bufs=1))
    qxp = ctx.enter_context(tc.tile_pool(name="qxp", bufs=1))
    qT_x = persist(qxp, [128, S], "qTx", NPAIR)
    # x2's qkv allocated before kvxp so kvxp can release first (LIFO)
    kvx2p = ctx.enter_context(tc.tile_pool(name="kvx2p", bufs=1))
    qT_x2 = persist(kvx2p, [128, S], "qTx2", NPAIR)
    kT_x2 = persist(kvx2p, [128, S], "kTx2", NPAIR)
    v_x2 = persist(kvx2p, [128, HEADS * D], "vx2", ST)
    # oT slots: tag "oTa" holds oT1 then oT3; "oTb" holds oT2
    otp = ctx.enter_context(tc.tile_pool(name="otp", bufs=1))
    oT1 = persist(otp, [128, S], "oTa", NPAIR)
    kvxp = tc.alloc_tile_pool(name="kvxp", bufs=1)
    kT_x = persist(kvxp, [128, S], "kTx", NPAIR)
    v_x = persist(kvxp, [128, HEADS * D], "vx", ST)
    wqp = tc.alloc_tile_pool(name="wqp", bufs=1)
    x2tp = tc.alloc_tile_pool(name="x2tp", bufs=1)
    x2T = x2tp.tile([128, KT * S], F16, name="x2T")
    # ---------------- phase 1 -----------------------------------------
    # DMA order: x first (transposes start ~4us in and keep the PE HAM
    # warm), then x2 (x2T transposes fill the PE while Wqkv streams),
    # then Wqkv, then bias/Wout (needed only by phase 3). Weight
    # fp32->fp16 conversion runs on ScalarE, which is idle until the
    # first attention exp; DVE keeps the PSUM-evac role.
    xnat = tc.alloc_tile_pool(name="xnat", bufs=4, side="right")
    wstage = tc.alloc_tile_pool(name="wstage", bufs=2, side="right")
    p1ps = tc.alloc_tile_pool(name="p1ps", bufs=2, space="PSUM")
    xtp = tc.alloc_tile_pool(name="xtp", bufs=1)
    xT = xtp.tile([128, KT * S], F16, name="xT")
    _emit_xt(c, x, xT, xnat, p1ps, "p1")
    _emit_xt(c, x2, x2T, xnat, p1ps, "p1")
    # Wqkv DMA'd in three column blocks, v first: the v projections can
    # then start as soon as the (much smaller) v-block lands instead of
    # waiting for the whole 7 MB matrix.
    c.wq16 = [wqp.tile([128, 3 * HIDDEN], F16, name=f"wq16{kt}",
                       tag=f"wq16{kt}") for kt in range(KT)]
    for base in (2 * HIDDEN, 0, HIDDEN):        # v, q, k blocks
        for kt in range(KT):
            f = wstage.tile([128, HIDDEN], F32, name="wqf", tag="wqf",
                            bufs=3)
            nc.sync.dma_start(
                f[:], wq[kt * 128:(kt + 1) * 128, base:base + HIDDEN])
            nc.scalar.copy(c.wq16[kt][:, base:base + HIDDEN], f[:])
    bo_bcast = bass.AP(tensor=bo.tensor, offset=bo.offset,
                       ap=[[0, 128]] + list(bo.ap))
    nc.sync.dma_start(c.bias_sb[:], bo_bcast)
    c.wout16 = []
    for ct in range(KT):
        f = wstage.tile([128, HIDDEN], F32, name="wof", tag="wof")
        nc.sync.dma_start(f[:], wo[ct * 128:(ct + 1) * 128, :])
        t = woutp.tile([128, HIDDEN], F16, name=f"wout{ct}", tag=f"wout{ct}")
        nc.scalar.copy(t[:], f[:])
        c.wout16.append(t)
    wstage.release()
    c.ysb = tc.alloc_tile_pool(name="ysb", bufs=2, side="right")
    _emit_qkv(c, xT, qT_x, kT_x, v_x, p1ps, "p1")
    xtp.release()
    p1ps.release()

    # ---- phase 2: attn(o1), aux = k2 + v2 ----------------------------
    auxp = tc.alloc_tile_pool(name="auxp", bufs=1, space="PSUM")
    AUXT = ["aux"]
    thunks2 = []
    _emit_qkv(c, x2T, qT_x2, kT_x2, v_x2, auxp, AUXT, thunks=thunks2,
              parts=("v", "k"))
    _attn(c, tc, qT_x, kT_x, v_x, oT1, thunks2)

    # ---- phase 3: attn(o3), aux = q2 + proj(y1) ----------------------
    oT3 = persist(otp, [128, S], "oTb", NPAIR)
    thunks3 = []
    _emit_qkv(c, x2T, qT_x2, kT_x2, v_x2, auxp, AUXT, thunks=thunks3,
              parts=("q",))
    _emit_proj(c, oT1, y1, auxp, AUXT, thunks=thunks3)
    _attn(c, tc, qT_x, kT_x2, v_x2, oT3, thunks3)
    x2tp.release()
    wqp.release()
    kvxp.release()

    # ---- phase 4: attn(o2), aux = proj(y3) ---------------------------
    oT2 = persist(otp, [128, S], "oTa", NPAIR)
    thunks4 = []
    _emit_proj(c, oT3, y3, auxp, AUXT, thunks=thunks4)
    _attn(c, tc, qT_x2, kT_x2, v_x2, oT2, thunks4)

    # ---- phase 5: proj(y2), with 4 psum banks to pipeline the tail ---
    auxp.release()
    p5ps = tc.alloc_tile_pool(name="p5ps", bufs=2, space="PSUM")
    _emit_proj(c, oT2, y2, p5ps, ["p5a", "p5b"])
    p5ps.release()
    c.ysb.release()
    xnat.release()


def build_bass():
    from contextlib import ExitStack
    nc = bacc.Bacc("TRN2", target_bir_lowering=False, debug=False,
                   num_devices=B)
    x = nc.dram_tensor("x", [S, HIDDEN], F32, kind="ExternalInput").ap()
    x2 = nc.dram_tensor("x2", [S, HIDDEN], F32, kind="ExternalInput").ap()
    wq = nc.dram_tensor("Wqkv", [HIDDEN, 3 * HIDDEN], F32,
                        kind="ExternalInput").ap()
    wo = nc.dram_tensor("Wout", [HIDDEN, HIDDEN], F32,
                        kind="ExternalInput").ap()
    bo = nc.dram_tensor("bout", [HIDDEN], F32, kind="ExternalInput").ap()
    y1 = nc.dram_tensor("y1", [S, HIDDEN], F32, kind="ExternalOutput").ap()
    y2 = nc.dram_tensor("y2", [S, HIDDEN], F32, kind="ExternalOutput").ap()
    y3 = nc.dram_tensor("y3", [S, HIDDEN], F32, kind="ExternalOutput").ap()
    with tile.TileContext(nc) as tc:
        with ExitStack() as ctx:
            build_kernel(ctx, tc, x, x2, wq, wo, bo, y1, y2, y3)
    nc.compile()
    return nc


_NC_CACHE = []


def kernel(x, x2, Wqkv, Wout, bout):
    if not _NC_CACHE:
        _NC_CACHE.append(build_bass())
    nc = _NC_CACHE[0]
    in_maps = [
        {"x": np.ascontiguousarray(x[b]), "x2": np.ascontiguousarray(x2[b]),
         "Wqkv": Wqkv, "Wout": Wout, "bout": bout}
        for b in range(B)
    ]
    res = run_bass_kernel_spmd(nc, in_maps, list(range(B)))
    y1 = np.stack([res.results[b]["y1"] for b in range(B)])
    y2 = np.stack([res.results[b]["y2"] for b in range(B)])
    y3 = np.stack([res.results[b]["y3"] for b in range(B)])
    return (y1, y2, y3)

